# revision 43
# baseline (speedup 1.0000x reference)
"""Trainium2 Bass kernel for BatteryMoEFlattenIntraCycleMoELayer.

out[b] = sum_{e in top2(b)} gate[b,e] * (x[b] @ W_e.T + bias_e),  cast to bf16

Strategy: expert-packed dispatch with host-side routing.

The gate-weighted top-2 dispatch decomposes into ~251 (sample, expert) tasks
of shape [L=100, F=900] @ [900, D=512].  The host computes the gating, packs
the L-rows of all tasks routed to the same expert into dense 128-row blocks,
and balances the ~200 blocks across 8 cores (nblk slots each).  Each core
runs nblk x 8 dense bf16 matmuls (PSUM-accumulated over K=901 in 8 k-tiles),
scaling by the gate at PSUM->SBUF eviction; the host gathers each sample's
two partial blocks and adds them.

DMA cost model (validated in TimelineSim): every DMACopy serializes ~630ns
on a single HWDGE descriptor-generation slot regardless of size, transfers
serialize at ~360GB/s on the DMA-engine pool, and completion is signalled
~900ns after transfer end.  The baseline's 69 per-core DMAs made HWDGE a
co-bottleneck with the PE (43us each); this version merges to ~28 DMAs:

  - W in k-major layout [128, KT, slots*D]: slot-0 k-pairs paced with
    wave 0; the non-slot-0 chunks trail ALL x chunks (at real HW transfer
    rates they otherwise push the x13-17 chunk past the PE's block-13
    deadline, stalling the PE and resetting its p-state ramp).
  - x as one 3D tile [128, nblk, KT*128]: wave 0 (blocks 0-7) loaded as
    k-pair slices across all 8 blocks (k-outer matmul order consumes them
    in arrival order); remaining blocks in ~5-block chunks.
  - outputs staged in [128, 2, D] pair tiles; all pair stores issue
    after the last load, so store transfers never queue ahead of x
    arrivals in the DMA-engine FIFO (HW transfers run ~24% slower than
    the modeled 360GB/s, thinning the sim's slack).
  - W-slot offsets as a single [1, nblk] i32 row, loaded outside the
    early-wave transfer FIFO (gate scales are pre-folded into x on the
    host, so no per-block scale data ships at all).

DMAs are emitted just-in-time (right before their first consumer's pass) so
tile-framework hazard tracking never creates a dependency on a later-arriving
transfer.  A short junk-matmul warmup keeps the PE clock ramping while the
first DMAs land.
"""

import numpy as np
import ml_dtypes
from contextlib import ExitStack

import concourse.bass as bass
import concourse.bacc as bacc
import concourse.mybir as mybir
import concourse.tile as tile
from concourse.bass_utils import run_bass_kernel_spmd

# problem shape (hardcoded per contract)
B, L, C, CURVE = 128, 100, 3, 300
F = C * CURVE            # 900
E, D, TOPK = 8, 512, 2
EPS = 1e-9

NCORES = 8
KT = 8                   # contraction tiles of 128 (900+bias row padded to 1024)
FP = KT * 128            # 1024
KTD = KT * D             # per-slot W span in the s-major layout
NBLK_MAX = 26            # block-slot cap; sum_e ceil(100*n_e/128) <= 207
SLOTS_MAX = 4            # expert W-slot cap per core (packer asserts this)
WAVE = 8                 # blocks in flight (one PSUM bank each)
WARMUP_MMS = 26          # junk matmuls covering the first-DMA latency

BF16 = mybir.dt.bfloat16
F32 = mybir.dt.float32
I32 = mybir.dt.int32

_BF = ml_dtypes.bfloat16

_NC_CACHE = {}


def _chunks(start, end, step):
    return [(a, min(a + step, end)) for a in range(start, end, step)]


def _emit_body(nc, tc, ctx, xh, wh, sc, hot, out, nblk, slots, nconst,
               shared=None, R=""):
    PE = mybir.EngineType.PE

    gp = ctx.enter_context(tc.tile_pool(name=f"{R}gating", bufs=1))
    wp = ctx.enter_context(tc.tile_pool(name=f"{R}wpool", bufs=1))
    xp = ctx.enter_context(tc.tile_pool(name=f"{R}xpool", bufs=1))
    pp = ctx.enter_context(tc.tile_pool(name=f"{R}psum", bufs=WAVE, space="PSUM"))
    op = ctx.enter_context(
        tc.tile_pool(name=f"{R}outp", bufs=NBLK_MAX // 2 + 2))

    sc_t = gp.tile([1, nblk], I32, name=f"{R}sct")
    w_t = wp.tile([128, KT, slots * D], BF16, name=f"{R}wt")
    x_t = xp.tile([128, nblk, KT * 128], BF16, name=f"{R}xt")
    ht = wp.tile([128, 2 * D + 2 * 256], BF16, name=f"{R}ht")
    junk = wp.tile([128, 128], BF16, name=f"{R}junk")

    psum_t = {}
    for j in range(WAVE):
        psum_t[j] = pp.tile([128, D], F32, tag="ps", name=f"{R}ps{j}")

    # PE warmup: junk matmuls into psum bank 0 keep the PE clock ramping
    # while the first DMAs land; block 0's start=True k0 overwrites the
    # bank.  N=128 so each costs ~107ns mid-ramp.  Full memset: garbage
    # bf16 can be NaN/Inf on real hardware.
    nc.vector.memset(junk, 0)
    for _ in range(WARMUP_MMS):
        nc.tensor.matmul(psum_t[0][:, 0:128], junk, junk,
                         start=True, stop=True)

    sp = nc.sync

    # ---- wave-0 leading loads.  The hot buffer carries W-slot0-k01 and
    # x k01 of blocks 0-1 in ONE 3KB/partition DMA (one HWDGE slot + one
    # transfer), so the first matmul starts ~4.0us in.  k01 of blocks 0-1
    # (lhsT) and of all const-rhs blocks (rhs) are consumed from ht; the
    # w_t/x_t regions they would occupy are never loaded.
    first_rep = shared is None or "offs" not in shared
    sp.dma_start(ht, hot[:, :])
    sp.dma_start(x_t[:, 2:4, 0:256], xh[:, 2:4, 0:256])
    sp.dma_start(x_t[:, 4:WAVE, 0:256], xh[:, 4:WAVE, 0:256])
    if first_rep and nconst < WAVE:
        # dynamic rhs inside wave 0: offsets must land before k0
        sp.dma_start(sc_t, sc[:, :])

    # W-slot offsets are rep-invariant: load them into PE registers once
    # (rep 0) and reuse across repeats -- per-rep loads exhaust the 54-reg
    # PE file at high repeat counts.
    offs = [None] * nblk if first_rep else shared["offs"]

    def load_offs():
        if nconst >= nblk or not first_rep:
            return
        _, offs1 = nc.values_load_multi_w_load_instructions(
            sc_t[0:1, nconst:nblk], engines=(PE,),
            min_val=0, max_val=(slots - 1) * D,
            skip_runtime_bounds_check=True)
        offs[nconst:] = list(offs1)
        if shared is not None:
            shared["offs"] = offs

    if nconst < WAVE:
        load_offs()

    def emit_mm(j, k, ps=None, n0=0, n1=D):
        if j < nconst:
            rhs = (ht[:, k * D + n0:k * D + n1] if k < 2
                   else w_t[:, k, n0:n1])
        else:
            # ds on the last dim with element offsets (slot * D), the
            # register-liveness-friendly form
            off = offs[j] if n0 == 0 else offs[j] + n0
            rhs = w_t[:, k, bass.ds(off, n1 - n0)]
        if j < 2 and k < 2:
            lhs = ht[:, 2 * D + j * 256 + k * 128:2 * D + j * 256 + (k + 1) * 128]
        else:
            lhs = x_t[:, j, k * 128:(k + 1) * 128]
        nc.tensor.matmul(
            psum_t[j] if ps is None else ps, lhs, rhs,
            start=(k == 0), stop=(k == KT - 1))

    pair_t = {}
    deferred = []    # (p, j_hi): pair stores issued after ALL loads so
    #                  store transfers never delay x arrivals in the
    #                  DMA-engine FIFO (HW transfers run slower than the
    #                  sim's 360GB/s; the slack is thinner than modeled)

    def emit_evict(j):
        # gates are pre-folded into x on the host, so eviction is a pure
        # f32->bf16 convert (tensor_scalar_mul by 1.0, no sc dependency)
        p, h = divmod(j, 2)
        if h == 0:
            pair_t[p] = op.tile([128, 2, D], BF16, tag="ot", name=f"{R}ot{j}")
        nc.vector.tensor_scalar_mul(pair_t[p][:, h, :], psum_t[j], 1.0)
        if h == 1 or j == nblk - 2:
            deferred.append((p, j))

    # ---- wave 0: k-outer over blocks 0-7, with per-pass loads emitted
    # between passes (just-in-time emission keeps hazard tracking exact and
    # paces one ~630ns HWDGE slot per instruction).
    # Non-slot-0 W is needed from block `nconst` (~19 typical, >= 8 by
    # packer guarantee when possible); when nconst is small, load it before
    # the bulk x chunks instead of after.
    wrest_early = nconst < 14
    # first bulk chunk split small: blocks 8-9 arrive ~3us earlier, widening
    # the thinnest post-wave-0 margin at real HW transfer rates
    if nblk > WAVE + 2:
        m = min(WAVE + 5, nblk)
        x_bulk = [(WAVE, WAVE + 2), (WAVE + 2, m)] + _chunks(m, nblk, 5)
    else:
        x_bulk = _chunks(WAVE, nblk, 5)
    wrest = []
    if slots > 1:
        wrest = [(0, KT // 2), (KT // 2, KT)]

    later_loads = []                     # emitted one per wave-0 pass
    for p in range(1, 4):                # k-pairs 23, 45, 67
        later_loads.append(("w0", (2 * p * D, (2 * p + 2) * D)))
        later_loads.append(("x0", (2 * p * 128, (2 * p + 2) * 128)))
    # w slot-0 k01 is consumed from ht by const-rhs blocks, but dynamic-rhs
    # blocks on cores whose slot-0 prefix exceeds the global nconst reach it
    # through w_t with offset 0 -- so load it (off the critical path).
    # At measured HW transfer rates the x13-17 chunk lands ~1.7us after
    # the PE's block-13 deadline when the W-rest chunks precede it in the
    # transfer FIFO.  W-rest isn't consumed until block nconst (>= 19
    # deadline ~37us), so in the normal case ALL x chunks go first and the
    # W-rest + dynamic-slot0 loads trail them.
    if wrest_early:
        bulk = ([("x", x_bulk[0])] +
                [("wr", rng) for rng in wrest] +
                [("w0", (0, 2 * D))] +
                [("x", rng) for rng in x_bulk[1:]])
    else:
        bulk = ([("x", rng) for rng in x_bulk] +
                [("wr", rng) for rng in wrest] +
                [("w0", (0, 2 * D))])
    later_loads += bulk

    def emit_load(item):
        eng = sp
        kind, (a, b) = item
        if kind == "w0":
            # a, b are column offsets within slot 0 across k-pairs: the
            # (p) pair loads k-tiles a//D..b//D of the slot-0 columns
            eng.dma_start(w_t[:, a // D:b // D, 0:D], wh[:, a // D:b // D, 0:D])
        elif kind == "wr":
            eng.dma_start(w_t[:, a:b, D:slots * D], wh[:, a:b, D:slots * D])
        elif kind == "x0":
            eng.dma_start(x_t[:, 0:WAVE, a:b], xh[:, 0:WAVE, a:b])
        else:
            eng.dma_start(x_t[:, a:b, :], xh[:, a:b, :])

    li = 0
    for k in range(KT):
        for j in range(WAVE):
            emit_mm(j, k)
        if li < len(later_loads):
            emit_load(later_loads[li])
            li += 1
        if k == 2 and first_rep and nconst >= WAVE:
            # tiny offsets row, kept out of the early-wave transfer FIFO
            sp.dma_start(sc_t, sc[:, :])
        if k == 3 and nconst >= WAVE:
            load_offs()
    # x chunks not yet emitted go out during the first steady blocks
    pending = later_loads[li:]

    # ---- steady state: evict the block whose PSUM bank is being recycled,
    # then run the next block k-inner.
    last = nblk - 1
    for j in range(WAVE, last):
        if pending:
            emit_load(pending.pop(0))
        emit_evict(j - WAVE)
        psum_t[j] = pp.tile([128, D], F32, tag="ps", name=f"{R}ps{j}")
        for k in range(KT):
            emit_mm(j, k)
    while pending:
        emit_load(pending.pop(0))

    # ---- final block: two half-width (N=256) matmul streams into TWO
    # psum banks, so half A's convert-evict + store overlap half B's
    # matmuls and the post-last-matmul chain is evict(392) -> issue ->
    # 182ns transfer -> sem.  The two recycled banks' drain evictions are
    # emitted first.
    H = D // 2
    emit_evict(last - WAVE)
    ps_a = pp.tile([128, H], F32, tag="ps", name=f"{R}psA")
    emit_evict(last - WAVE + 1)
    ps_b = pp.tile([128, H], F32, tag="ps", name=f"{R}psB")
    for k in range(KT):
        emit_mm(last, k, ps=ps_a, n0=0, n1=H)
    for j in range(last - WAVE + 2, last):
        emit_evict(j)
    for (p, j) in deferred:
        if j % 2 == 1:
            nc.scalar.dma_start(out[:, j - 1:j + 1, :], pair_t[p])
        else:
            # nblk even leaves block nblk-2 unpaired; store it solo
            nc.scalar.dma_start(out[:, j:j + 1, :], pair_t[p][:, 0:1, :])
    ot_l = op.tile([128, 1, D], BF16, tag="ot", name=f"{R}otl")
    nc.vector.tensor_scalar_mul(ot_l[:, 0, 0:H], ps_a, 1.0)
    nc.scalar.dma_start(out[:, last:last + 1, 0:H], ot_l[:, :, 0:H])
    for k in range(KT):
        emit_mm(last, k, ps=ps_b, n0=H, n1=D)
    nc.vector.tensor_scalar_mul(ot_l[:, 0, H:D], ps_b, 1.0)
    sp.dma_start(out[:, last:last + 1, H:D], ot_l[:, :, H:D])


def _build_nc(repeats=1, nblk=NBLK_MAX, slots=SLOTS_MAX, nconst=0):
    nc = bacc.Bacc("TRN2", target_bir_lowering=False)

    xh = nc.declare_dram_parameter("xh", [128, nblk, KT * 128], BF16,
                                   isOutput=False)
    wh = nc.declare_dram_parameter("wh", [128, KT, slots * D], BF16,
                                   isOutput=False)
    sc = nc.declare_dram_parameter("sc", [1, nblk], I32, isOutput=False)
    hot = nc.declare_dram_parameter("hot", [128, 2 * D + 2 * 256], BF16,
                                    isOutput=False)
    out = nc.declare_dram_parameter("out", [128, nblk, D], BF16, isOutput=True)

    with tile.TileContext(nc) as tc, ExitStack() as ctx:
        shared = {}
        for rep in range(repeats):
            R = f"r{rep}_" if repeats > 1 else ""
            with ExitStack() as rctx:
                _emit_body(nc, tc, rctx, xh, wh, sc, hot, out,
                           nblk, slots, nconst, shared=shared, R=R)

    nc.compile()
    return nc


def get_nc(repeats=1, nblk=NBLK_MAX, slots=SLOTS_MAX, nconst=0):
    key = ("nc", repeats, nblk, slots, nconst)
    if key not in _NC_CACHE:
        _NC_CACHE[key] = _build_nc(repeats, nblk, slots, nconst)
    return _NC_CACHE[key]


def _host_gates(logits, moe_masks):
    """Reference gating on host -> per-sample (g0, g1), (e0, e1)."""
    lg = np.asarray(logits, np.float64)
    mk = (np.asarray(moe_masks, np.int64) == 1).astype(np.float64)
    p = np.exp(lg - lg.max(axis=1, keepdims=True))
    p /= p.sum(axis=1, keepdims=True)
    g = p * mk                                              # [B, E]
    idx = np.argsort(-g, axis=1, kind="stable")[:, :TOPK]   # top-2 indices
    gv = np.take_along_axis(g, idx, axis=1)                 # [B, 2]
    gv = gv / (gv.sum(axis=1, keepdims=True) + EPS)         # renormalize
    return gv.astype(np.float32), idx.astype(np.int64)


def _assign_blocks(nblocks_per_expert, nblk):
    """Distribute each expert's blocks over 8 cores of nblk slots,
    minimizing distinct experts per core.  Phase 1: every expert gets its
    own (empty) core, largest first, filled up to nblk.  Phase 2: leftover
    pieces go to the cores with the fewest distinct experts / most room."""
    cap = [nblk] * NCORES
    experts_on = [[] for _ in range(NCORES)]   # ordered distinct experts
    placed = [[] for _ in range(NCORES)]       # (expert, nblocks)

    def put(c, e, take):
        cap[c] -= take
        if e not in experts_on[c]:
            experts_on[c].append(e)
        placed[c].append((e, take))

    order = [e for e in sorted(range(E), key=lambda e: -nblocks_per_expert[e])
             if nblocks_per_expert[e] > 0]
    leftovers = []
    nxt = 0
    for e in order:
        rem = nblocks_per_expert[e]
        if nxt < NCORES:
            take = min(rem, nblk)
            put(nxt, e, take)
            nxt += 1
            rem -= take
        if rem:
            leftovers.append((e, rem))
    leftovers.sort(key=lambda x: -x[1])
    for e, rem in leftovers:
        while rem > 0:
            cands = [c for c in range(NCORES) if cap[c] > 0]
            cands.sort(key=lambda c: (e not in experts_on[c],
                                      len(experts_on[c]), -cap[c]))
            c = cands[0]
            take = min(rem, cap[c])
            put(c, e, take)
            rem -= take
    nslots = max(len(x) for x in experts_on)
    assert nslots <= SLOTS_MAX, (
        f"packing needs {nslots} experts on one core > {SLOTS_MAX}")
    return placed, experts_on, max(2, nslots)


def _prep_w_full(W, b):
    """-> [E, KT, 128, D] f32 k-tiled transposed-padded weights."""
    wt = np.zeros((E, FP, D), np.float32)
    wt[:, :F, :] = np.asarray(W, np.float32).transpose(0, 2, 1)
    wt[:, F, :] = np.asarray(b, np.float32)
    return wt.reshape(E, KT, 128, D)


def make_in_maps(cycle_curve_data, logits, moe_masks, W, b):
    gv, idx = _host_gates(logits, moe_masks)

    # per-expert routed sample lists (zero-gate picks contribute exactly 0
    # and are dropped from dispatch; their combine position points at a
    # guaranteed-zero pad row)
    samples_e = [[] for _ in range(E)]     # (sample, gate)
    pick_pos = {}                          # (b, i) -> (expert, rank) | None
    for bb in range(B):
        for i in range(TOPK):
            e = int(idx[bb, i])
            g = float(gv[bb, i])
            if g == 0.0:
                pick_pos[(bb, i)] = None
                continue
            pick_pos[(bb, i)] = (e, len(samples_e[e]))
            samples_e[e].append((bb, g))
    n_e = [len(s) for s in samples_e]
    B_e = [int(np.ceil(L * n / 128)) if n else 0 for n in n_e]
    nblk = max(WAVE, int(np.ceil(sum(B_e) / NCORES)))
    assert nblk <= NBLK_MAX

    placed, _, slots = _assign_blocks(B_e, nblk)

    # Per-core block order: the core's largest expert becomes W-slot 0 and
    # its blocks (plus any pad blocks, which are also slot-0/offset-0) come
    # first, so a compile-time-constant rhs covers the first nconst blocks.
    experts_on = [[] for _ in range(NCORES)]
    core_blocks = [[] for _ in range(NCORES)]  # expert id per slot, -1 pad
    nconst = nblk
    for c in range(NCORES):
        cnt = {}
        for (e, take) in placed[c]:
            cnt[e] = cnt.get(e, 0) + take
        exps = sorted(cnt, key=lambda e: -cnt[e])
        experts_on[c] = exps
        npads = nblk - sum(cnt.values())
        if exps:
            seq = [exps[0]] * cnt[exps[0]] + [-1] * npads
            for e in exps[1:]:
                seq += [e] * cnt[e]
            nconst = min(nconst, cnt[exps[0]] + npads)
        else:
            seq = [-1] * nblk
        core_blocks[c] = seq

    # global row stream per expert -> (core, slot j, partition m) positions
    # flat position space: core*nblk*128 + j*128 + m
    expert_rowpos = {}                     # e -> int64 [100*n_e]
    next_blk_of = [0] * E
    expert_block_flat = [np.empty(B_e[e], np.int64) for e in range(E)]
    for c in range(NCORES):
        for j, e in enumerate(core_blocks[c]):
            if e >= 0:
                expert_block_flat[e][next_blk_of[e]] = c * nblk + j
                next_blk_of[e] += 1
    for e in range(E):
        if n_e[e] == 0:
            continue
        r = np.arange(L * n_e[e], dtype=np.int64)
        expert_rowpos[e] = expert_block_flat[e][r // 128] * 128 + r % 128

    # ---- pack x: xr[(b,l), f] = x row-major, padded to 1024 with ones@900
    # (kept f32; the per-row gate is folded in at the per-core gather)
    x = np.asarray(cycle_curve_data, np.float32).reshape(B, L, F)
    xr = np.zeros((B * L, FP), np.float32)
    xr[:, :F] = x.reshape(B * L, F)
    xr[:, F] = 1.0

    # per-core row index [nblk*128] into xr (pad rows -> 0 with scale 0)
    rowidx = np.zeros((NCORES, nblk * 128), np.int64)
    scales = np.zeros((NCORES, nblk * 128), np.float32)
    for e in range(E):
        if n_e[e] == 0:
            continue
        src = np.empty(L * n_e[e], np.int64)    # xr row ids of this stream
        gts = np.empty(L * n_e[e], np.float32)
        for r, (bb, g) in enumerate(samples_e[e]):
            src[r * L:(r + 1) * L] = np.arange(bb * L, (bb + 1) * L)
            gts[r * L:(r + 1) * L] = g
        pos = expert_rowpos[e]
        c = pos // (nblk * 128)
        m = pos % (nblk * 128)
        rowidx[c, m] = src
        scales[c, m] = gts

    # gather + transpose to device layout
    wt = _prep_w_full(W, b)
    in_maps = []
    for c in range(NCORES):
        xb = (xr[rowidx[c]] * scales[c][:, None]).astype(_BF)
        xb = xb.reshape(nblk, 128, KT, 128)         # [j, m, k, p]
        xhc = np.ascontiguousarray(xb.transpose(3, 0, 2, 1)).reshape(
            128, nblk, KT * 128)
        # W s-major k-inner: wh[p, s*KTD + k*D + c] = wt[e_s][k, p, c]
        whc = np.zeros((slots, KT, 128, D), np.float32)
        for s, e in enumerate(experts_on[c]):
            whc[s] = wt[e]
        whc = np.ascontiguousarray(whc.transpose(2, 1, 0, 3)).reshape(
            128, KT, slots * D).astype(_BF)
        slot_of = {e: s for s, e in enumerate(experts_on[c])}
        ohv = np.zeros(nblk, np.int32)
        for j, e in enumerate(core_blocks[c]):
            ohv[j] = slot_of[e] * D if e >= 0 else 0
        scc = ohv.reshape(1, nblk)
        hotc = np.concatenate(
            [whc[:, 0, 0:D], whc[:, 1, 0:D],
             xhc[:, 0:2, 0:256].reshape(128, 512)], axis=1)
        in_maps.append({"xh": xhc, "wh": whc, "sc": scc,
                        "hot": np.ascontiguousarray(hotc)})

    # combine positions for the host-side gather-add; dropped picks point
    # at a pad row (scale 0 -> exact zero)
    zeros_flat = np.flatnonzero(scales.reshape(-1) == 0.0)
    zeropos = int(zeros_flat[0]) if len(zeros_flat) else 0
    pos = np.empty((TOPK, B, L), np.int64)
    for bb in range(B):
        for i in range(TOPK):
            pp_ = pick_pos[(bb, i)]
            if pp_ is None:
                pos[i, bb] = zeropos
            else:
                e, rank = pp_
                pos[i, bb] = expert_rowpos[e][rank * L:(rank + 1) * L]
    return in_maps, pos, nblk, slots, nconst


LAST_RESULT = None


def kernel(cycle_curve_data, logits, moe_masks, W, b):
    global LAST_RESULT
    in_maps, pos, nblk, slots, nconst = make_in_maps(
        cycle_curve_data, logits, moe_masks, W, b)
    nc = get_nc(nblk=nblk, slots=slots, nconst=nconst)
    res = run_bass_kernel_spmd(nc, in_maps, core_ids=list(range(NCORES)))
    LAST_RESULT = res
    # out is [128m, nblk, D] per core; flat position space (c, j, m)
    flat = np.concatenate(
        [np.asarray(r["out"]).astype(np.float32).transpose(1, 0, 2)
         .reshape(nblk * 128, D) for r in res.results], axis=0)
    out = flat[pos[0].reshape(-1)] + flat[pos[1].reshape(-1)]
    return out.reshape(B, L, D).astype(_BF)


# revision 44
# speedup vs baseline: 1.0036x; 1.0036x over previous
"""Trainium2 Bass kernel for BatteryMoEFlattenIntraCycleMoELayer.

out[b] = sum_{e in top2(b)} gate[b,e] * (x[b] @ W_e.T + bias_e),  cast to bf16

Strategy: expert-packed dispatch with host-side routing.

The gate-weighted top-2 dispatch decomposes into ~251 (sample, expert) tasks
of shape [L=100, F=900] @ [900, D=512].  The host computes the gating, packs
the L-rows of all tasks routed to the same expert into dense 128-row blocks,
and balances the ~200 blocks across 8 cores (nblk slots each).  Each core
runs nblk x 8 dense bf16 matmuls (PSUM-accumulated over K=901 in 8 k-tiles),
scaling by the gate at PSUM->SBUF eviction; the host gathers each sample's
two partial blocks and adds them.

DMA cost model (validated in TimelineSim): every DMACopy serializes ~630ns
on a single HWDGE descriptor-generation slot regardless of size, transfers
serialize at ~360GB/s on the DMA-engine pool, and completion is signalled
~900ns after transfer end.  The baseline's 69 per-core DMAs made HWDGE a
co-bottleneck with the PE (43us each); this version merges to ~28 DMAs:

  - W in k-major layout [128, KT, slots*D]: slot-0 k-pairs paced with
    wave 0; the non-slot-0 chunks trail ALL x chunks (at real HW transfer
    rates they otherwise push the x13-17 chunk past the PE's block-13
    deadline, stalling the PE and resetting its p-state ramp).
  - x as one 3D tile [128, nblk, KT*128]: wave 0 (blocks 0-7) loaded as
    k-pair slices across all 8 blocks (k-outer matmul order consumes them
    in arrival order); remaining blocks in ~5-block chunks.
  - outputs staged in [128, 2, D] pair tiles; all pair stores issue
    after the last load, so store transfers never queue ahead of x
    arrivals in the DMA-engine FIFO (HW transfers run ~24% slower than
    the modeled 360GB/s, thinning the sim's slack).
  - W-slot offsets as a single [1, nblk] i32 row, loaded outside the
    early-wave transfer FIFO (gate scales are pre-folded into x on the
    host, so no per-block scale data ships at all).

DMAs are emitted just-in-time (right before their first consumer's pass) so
tile-framework hazard tracking never creates a dependency on a later-arriving
transfer.  A short junk-matmul warmup keeps the PE clock ramping while the
first DMAs land.
"""

import numpy as np
import ml_dtypes
from contextlib import ExitStack

import concourse.bass as bass
import concourse.bacc as bacc
import concourse.mybir as mybir
import concourse.tile as tile
from concourse.bass_utils import run_bass_kernel_spmd

# problem shape (hardcoded per contract)
B, L, C, CURVE = 128, 100, 3, 300
F = C * CURVE            # 900
E, D, TOPK = 8, 512, 2
EPS = 1e-9

NCORES = 8
KT = 8                   # contraction tiles of 128 (900+bias row padded to 1024)
FP = KT * 128            # 1024
KTD = KT * D             # per-slot W span in the s-major layout
NBLK_MAX = 26            # block-slot cap; sum_e ceil(100*n_e/128) <= 207
SLOTS_MAX = 4            # expert W-slot cap per core (packer asserts this)
WAVE = 8                 # blocks in flight (one PSUM bank each)
WARMUP_MMS = 26          # junk matmuls covering the first-DMA latency

BF16 = mybir.dt.bfloat16
F32 = mybir.dt.float32
I32 = mybir.dt.int32

_BF = ml_dtypes.bfloat16

_NC_CACHE = {}


def _chunks(start, end, step):
    return [(a, min(a + step, end)) for a in range(start, end, step)]


def _emit_body(nc, tc, ctx, xh, wh, sc, hot, out, nblk, slots, nconst,
               shared=None, R=""):
    PE = mybir.EngineType.PE

    gp = ctx.enter_context(tc.tile_pool(name=f"{R}gating", bufs=1))
    wp = ctx.enter_context(tc.tile_pool(name=f"{R}wpool", bufs=1))
    xp = ctx.enter_context(tc.tile_pool(name=f"{R}xpool", bufs=1))
    pp = ctx.enter_context(tc.tile_pool(name=f"{R}psum", bufs=WAVE, space="PSUM"))
    op = ctx.enter_context(
        tc.tile_pool(name=f"{R}outp", bufs=NBLK_MAX // 2 + 2))

    sc_t = gp.tile([1, nblk], I32, name=f"{R}sct")
    w_t = wp.tile([128, KT, slots * D], BF16, name=f"{R}wt")
    x_t = xp.tile([128, nblk, KT * 128], BF16, name=f"{R}xt")
    ht = wp.tile([128, 2 * D + 2 * 256], BF16, name=f"{R}ht")
    junk = wp.tile([128, 128], BF16, name=f"{R}junk")

    psum_t = {}
    for j in range(WAVE):
        psum_t[j] = pp.tile([128, D], F32, tag="ps", name=f"{R}ps{j}")

    # PE warmup: junk matmuls into psum bank 0 keep the PE clock ramping
    # while the first DMAs land; block 0's start=True k0 overwrites the
    # bank.  N=128 so each costs ~107ns mid-ramp.  Full memset: garbage
    # bf16 can be NaN/Inf on real hardware.
    nc.vector.memset(junk, 0)
    for _ in range(WARMUP_MMS):
        nc.tensor.matmul(psum_t[0][:, 0:128], junk, junk,
                         start=True, stop=True)

    sp = nc.sync

    # ---- wave-0 leading loads.  The hot buffer carries W-slot0-k01 and
    # x k01 of blocks 0-1 in ONE 3KB/partition DMA (one HWDGE slot + one
    # transfer), so the first matmul starts ~4.0us in.  k01 of blocks 0-1
    # (lhsT) and of all const-rhs blocks (rhs) are consumed from ht; the
    # w_t/x_t regions they would occupy are never loaded.
    first_rep = shared is None or "offs" not in shared
    sp.dma_start(ht, hot[:, :])
    sp.dma_start(x_t[:, 2:4, 0:256], xh[:, 2:4, 0:256])
    sp.dma_start(x_t[:, 4:WAVE, 0:256], xh[:, 4:WAVE, 0:256])
    if first_rep and nconst < WAVE:
        # dynamic rhs inside wave 0: offsets must land before k0
        sp.dma_start(sc_t, sc[:, :])

    # W-slot offsets are rep-invariant: load them into PE registers once
    # (rep 0) and reuse across repeats -- per-rep loads exhaust the 54-reg
    # PE file at high repeat counts.
    offs = [None] * nblk if first_rep else shared["offs"]

    def load_offs():
        if nconst >= nblk or not first_rep:
            return
        _, offs1 = nc.values_load_multi_w_load_instructions(
            sc_t[0:1, nconst:nblk], engines=(PE,),
            min_val=0, max_val=(slots - 1) * D,
            skip_runtime_bounds_check=True)
        offs[nconst:] = list(offs1)
        if shared is not None:
            shared["offs"] = offs

    if nconst < WAVE:
        load_offs()

    def emit_mm(j, k, ps=None, n0=0, n1=D):
        if j < nconst:
            rhs = (ht[:, k * D + n0:k * D + n1] if k < 2
                   else w_t[:, k, n0:n1])
        else:
            # ds on the last dim with element offsets (slot * D), the
            # register-liveness-friendly form
            off = offs[j] if n0 == 0 else offs[j] + n0
            rhs = w_t[:, k, bass.ds(off, n1 - n0)]
        if j < 2 and k < 2:
            lhs = ht[:, 2 * D + j * 256 + k * 128:2 * D + j * 256 + (k + 1) * 128]
        else:
            lhs = x_t[:, j, k * 128:(k + 1) * 128]
        nc.tensor.matmul(
            psum_t[j] if ps is None else ps, lhs, rhs,
            start=(k == 0), stop=(k == KT - 1))

    pair_t = {}
    deferred = []    # (p, j_hi): pair stores issued after ALL loads so
    #                  store transfers never delay x arrivals in the
    #                  DMA-engine FIFO (HW transfers run slower than the
    #                  sim's 360GB/s; the slack is thinner than modeled)

    def emit_evict(j):
        # gates are pre-folded into x on the host, so eviction is a pure
        # f32->bf16 convert (tensor_scalar_mul by 1.0, no sc dependency)
        p, h = divmod(j, 2)
        if h == 0:
            pair_t[p] = op.tile([128, 2, D], BF16, tag="ot", name=f"{R}ot{j}")
        nc.vector.tensor_scalar_mul(pair_t[p][:, h, :], psum_t[j], 1.0)
        if h == 1 or j == nblk - 2:
            deferred.append((p, j))

    # ---- wave 0: k-outer over blocks 0-7, with per-pass loads emitted
    # between passes (just-in-time emission keeps hazard tracking exact and
    # paces one ~630ns HWDGE slot per instruction).
    # Non-slot-0 W is needed from block `nconst` (~19 typical, >= 8 by
    # packer guarantee when possible); when nconst is small, load it before
    # the bulk x chunks instead of after.
    wrest_early = nconst < 14
    x_bulk = _chunks(WAVE, nblk, 5)
    wrest = []
    if slots > 1:
        wrest = [(0, KT // 2), (KT // 2, KT)]

    later_loads = []                     # emitted one per wave-0 pass
    for p in range(1, 4):                # k-pairs 23, 45, 67
        later_loads.append(("w0", (2 * p * D, (2 * p + 2) * D)))
        later_loads.append(("x0", (2 * p * 128, (2 * p + 2) * 128)))
    # w slot-0 k01 is consumed from ht by const-rhs blocks, but dynamic-rhs
    # blocks on cores whose slot-0 prefix exceeds the global nconst reach it
    # through w_t with offset 0 -- so load it (off the critical path).
    # At measured HW transfer rates the x13-17 chunk lands ~1.7us after
    # the PE's block-13 deadline when the W-rest chunks precede it in the
    # transfer FIFO.  W-rest isn't consumed until block nconst (>= 19
    # deadline ~37us), so in the normal case ALL x chunks go first and the
    # W-rest + dynamic-slot0 loads trail them.
    if wrest_early:
        bulk = ([("x", x_bulk[0])] +
                [("wr", rng) for rng in wrest] +
                [("w0", (0, 2 * D))] +
                [("x", rng) for rng in x_bulk[1:]])
    else:
        bulk = ([("x", rng) for rng in x_bulk] +
                [("wr", rng) for rng in wrest] +
                [("w0", (0, 2 * D))])
    later_loads += bulk

    def emit_load(item):
        eng = sp
        kind, (a, b) = item
        if kind == "w0":
            # a, b are column offsets within slot 0 across k-pairs: the
            # (p) pair loads k-tiles a//D..b//D of the slot-0 columns
            eng.dma_start(w_t[:, a // D:b // D, 0:D], wh[:, a // D:b // D, 0:D])
        elif kind == "wr":
            eng.dma_start(w_t[:, a:b, D:slots * D], wh[:, a:b, D:slots * D])
        elif kind == "x0":
            eng.dma_start(x_t[:, 0:WAVE, a:b], xh[:, 0:WAVE, a:b])
        else:
            eng.dma_start(x_t[:, a:b, :], xh[:, a:b, :])

    li = 0
    for k in range(KT):
        for j in range(WAVE):
            emit_mm(j, k)
        if li < len(later_loads):
            emit_load(later_loads[li])
            li += 1
        if k == 2 and first_rep and nconst >= WAVE:
            # tiny offsets row, kept out of the early-wave transfer FIFO
            sp.dma_start(sc_t, sc[:, :])
        if k == 3 and nconst >= WAVE:
            load_offs()
    # x chunks not yet emitted go out during the first steady blocks
    pending = later_loads[li:]

    # ---- steady state: evict the block whose PSUM bank is being recycled,
    # then run the next block k-inner.
    last = nblk - 1
    for j in range(WAVE, last):
        if pending:
            emit_load(pending.pop(0))
        emit_evict(j - WAVE)
        psum_t[j] = pp.tile([128, D], F32, tag="ps", name=f"{R}ps{j}")
        for k in range(KT):
            emit_mm(j, k)
    while pending:
        emit_load(pending.pop(0))

    # ---- final block: two half-width (N=256) matmul streams into TWO
    # psum banks, so half A's convert-evict + store overlap half B's
    # matmuls and the post-last-matmul chain is evict(392) -> issue ->
    # 182ns transfer -> sem.  The two recycled banks' drain evictions are
    # emitted first.
    H = D // 2
    emit_evict(last - WAVE)
    ps_a = pp.tile([128, H], F32, tag="ps", name=f"{R}psA")
    emit_evict(last - WAVE + 1)
    ps_b = pp.tile([128, H], F32, tag="ps", name=f"{R}psB")
    for k in range(KT):
        emit_mm(last, k, ps=ps_a, n0=0, n1=H)
    for j in range(last - WAVE + 2, last):
        emit_evict(j)
    for (p, j) in deferred:
        if j % 2 == 1:
            nc.scalar.dma_start(out[:, j - 1:j + 1, :], pair_t[p])
        else:
            # nblk even leaves block nblk-2 unpaired; store it solo
            nc.scalar.dma_start(out[:, j:j + 1, :], pair_t[p][:, 0:1, :])
    ot_l = op.tile([128, 1, D], BF16, tag="ot", name=f"{R}otl")
    nc.vector.tensor_scalar_mul(ot_l[:, 0, 0:H], ps_a, 1.0)
    nc.scalar.dma_start(out[:, last:last + 1, 0:H], ot_l[:, :, 0:H])
    for k in range(KT):
        emit_mm(last, k, ps=ps_b, n0=H, n1=D)
    nc.vector.tensor_scalar_mul(ot_l[:, 0, H:D], ps_b, 1.0)
    sp.dma_start(out[:, last:last + 1, H:D], ot_l[:, :, H:D])


def _build_nc(repeats=1, nblk=NBLK_MAX, slots=SLOTS_MAX, nconst=0):
    nc = bacc.Bacc("TRN2", target_bir_lowering=False)

    xh = nc.declare_dram_parameter("xh", [128, nblk, KT * 128], BF16,
                                   isOutput=False)
    wh = nc.declare_dram_parameter("wh", [128, KT, slots * D], BF16,
                                   isOutput=False)
    sc = nc.declare_dram_parameter("sc", [1, nblk], I32, isOutput=False)
    hot = nc.declare_dram_parameter("hot", [128, 2 * D + 2 * 256], BF16,
                                    isOutput=False)
    out = nc.declare_dram_parameter("out", [128, nblk, D], BF16, isOutput=True)

    with tile.TileContext(nc) as tc, ExitStack() as ctx:
        shared = {}
        for rep in range(repeats):
            R = f"r{rep}_" if repeats > 1 else ""
            with ExitStack() as rctx:
                _emit_body(nc, tc, rctx, xh, wh, sc, hot, out,
                           nblk, slots, nconst, shared=shared, R=R)

    nc.compile()
    return nc


def get_nc(repeats=1, nblk=NBLK_MAX, slots=SLOTS_MAX, nconst=0):
    key = ("nc", repeats, nblk, slots, nconst)
    if key not in _NC_CACHE:
        _NC_CACHE[key] = _build_nc(repeats, nblk, slots, nconst)
    return _NC_CACHE[key]


def _host_gates(logits, moe_masks):
    """Reference gating on host -> per-sample (g0, g1), (e0, e1)."""
    lg = np.asarray(logits, np.float64)
    mk = (np.asarray(moe_masks, np.int64) == 1).astype(np.float64)
    p = np.exp(lg - lg.max(axis=1, keepdims=True))
    p /= p.sum(axis=1, keepdims=True)
    g = p * mk                                              # [B, E]
    idx = np.argsort(-g, axis=1, kind="stable")[:, :TOPK]   # top-2 indices
    gv = np.take_along_axis(g, idx, axis=1)                 # [B, 2]
    gv = gv / (gv.sum(axis=1, keepdims=True) + EPS)         # renormalize
    return gv.astype(np.float32), idx.astype(np.int64)


def _assign_blocks(nblocks_per_expert, nblk):
    """Distribute each expert's blocks over 8 cores of nblk slots,
    minimizing distinct experts per core.  Phase 1: every expert gets its
    own (empty) core, largest first, filled up to nblk.  Phase 2: leftover
    pieces go to the cores with the fewest distinct experts / most room."""
    cap = [nblk] * NCORES
    experts_on = [[] for _ in range(NCORES)]   # ordered distinct experts
    placed = [[] for _ in range(NCORES)]       # (expert, nblocks)

    def put(c, e, take):
        cap[c] -= take
        if e not in experts_on[c]:
            experts_on[c].append(e)
        placed[c].append((e, take))

    order = [e for e in sorted(range(E), key=lambda e: -nblocks_per_expert[e])
             if nblocks_per_expert[e] > 0]
    leftovers = []
    nxt = 0
    for e in order:
        rem = nblocks_per_expert[e]
        if nxt < NCORES:
            take = min(rem, nblk)
            put(nxt, e, take)
            nxt += 1
            rem -= take
        if rem:
            leftovers.append((e, rem))
    leftovers.sort(key=lambda x: -x[1])
    for e, rem in leftovers:
        while rem > 0:
            cands = [c for c in range(NCORES) if cap[c] > 0]
            cands.sort(key=lambda c: (e not in experts_on[c],
                                      len(experts_on[c]), -cap[c]))
            c = cands[0]
            take = min(rem, cap[c])
            put(c, e, take)
            rem -= take
    nslots = max(len(x) for x in experts_on)
    assert nslots <= SLOTS_MAX, (
        f"packing needs {nslots} experts on one core > {SLOTS_MAX}")
    return placed, experts_on, max(2, nslots)


def _prep_w_full(W, b):
    """-> [E, KT, 128, D] f32 k-tiled transposed-padded weights."""
    wt = np.zeros((E, FP, D), np.float32)
    wt[:, :F, :] = np.asarray(W, np.float32).transpose(0, 2, 1)
    wt[:, F, :] = np.asarray(b, np.float32)
    return wt.reshape(E, KT, 128, D)


def make_in_maps(cycle_curve_data, logits, moe_masks, W, b):
    gv, idx = _host_gates(logits, moe_masks)

    # per-expert routed sample lists (zero-gate picks contribute exactly 0
    # and are dropped from dispatch; their combine position points at a
    # guaranteed-zero pad row)
    samples_e = [[] for _ in range(E)]     # (sample, gate)
    pick_pos = {}                          # (b, i) -> (expert, rank) | None
    for bb in range(B):
        for i in range(TOPK):
            e = int(idx[bb, i])
            g = float(gv[bb, i])
            if g == 0.0:
                pick_pos[(bb, i)] = None
                continue
            pick_pos[(bb, i)] = (e, len(samples_e[e]))
            samples_e[e].append((bb, g))
    n_e = [len(s) for s in samples_e]
    B_e = [int(np.ceil(L * n / 128)) if n else 0 for n in n_e]
    nblk = max(WAVE, int(np.ceil(sum(B_e) / NCORES)))
    assert nblk <= NBLK_MAX

    placed, _, slots = _assign_blocks(B_e, nblk)

    # Per-core block order: the core's largest expert becomes W-slot 0 and
    # its blocks (plus any pad blocks, which are also slot-0/offset-0) come
    # first, so a compile-time-constant rhs covers the first nconst blocks.
    experts_on = [[] for _ in range(NCORES)]
    core_blocks = [[] for _ in range(NCORES)]  # expert id per slot, -1 pad
    nconst = nblk
    for c in range(NCORES):
        cnt = {}
        for (e, take) in placed[c]:
            cnt[e] = cnt.get(e, 0) + take
        exps = sorted(cnt, key=lambda e: -cnt[e])
        experts_on[c] = exps
        npads = nblk - sum(cnt.values())
        if exps:
            seq = [exps[0]] * cnt[exps[0]] + [-1] * npads
            for e in exps[1:]:
                seq += [e] * cnt[e]
            nconst = min(nconst, cnt[exps[0]] + npads)
        else:
            seq = [-1] * nblk
        core_blocks[c] = seq

    # global row stream per expert -> (core, slot j, partition m) positions
    # flat position space: core*nblk*128 + j*128 + m
    expert_rowpos = {}                     # e -> int64 [100*n_e]
    next_blk_of = [0] * E
    expert_block_flat = [np.empty(B_e[e], np.int64) for e in range(E)]
    for c in range(NCORES):
        for j, e in enumerate(core_blocks[c]):
            if e >= 0:
                expert_block_flat[e][next_blk_of[e]] = c * nblk + j
                next_blk_of[e] += 1
    for e in range(E):
        if n_e[e] == 0:
            continue
        r = np.arange(L * n_e[e], dtype=np.int64)
        expert_rowpos[e] = expert_block_flat[e][r // 128] * 128 + r % 128

    # ---- pack x: xr[(b,l), f] = x row-major, padded to 1024 with ones@900
    # (kept f32; the per-row gate is folded in at the per-core gather)
    x = np.asarray(cycle_curve_data, np.float32).reshape(B, L, F)
    xr = np.zeros((B * L, FP), np.float32)
    xr[:, :F] = x.reshape(B * L, F)
    xr[:, F] = 1.0

    # per-core row index [nblk*128] into xr (pad rows -> 0 with scale 0)
    rowidx = np.zeros((NCORES, nblk * 128), np.int64)
    scales = np.zeros((NCORES, nblk * 128), np.float32)
    for e in range(E):
        if n_e[e] == 0:
            continue
        src = np.empty(L * n_e[e], np.int64)    # xr row ids of this stream
        gts = np.empty(L * n_e[e], np.float32)
        for r, (bb, g) in enumerate(samples_e[e]):
            src[r * L:(r + 1) * L] = np.arange(bb * L, (bb + 1) * L)
            gts[r * L:(r + 1) * L] = g
        pos = expert_rowpos[e]
        c = pos // (nblk * 128)
        m = pos % (nblk * 128)
        rowidx[c, m] = src
        scales[c, m] = gts

    # gather + transpose to device layout
    wt = _prep_w_full(W, b)
    in_maps = []
    for c in range(NCORES):
        xb = (xr[rowidx[c]] * scales[c][:, None]).astype(_BF)
        xb = xb.reshape(nblk, 128, KT, 128)         # [j, m, k, p]
        xhc = np.ascontiguousarray(xb.transpose(3, 0, 2, 1)).reshape(
            128, nblk, KT * 128)
        # W s-major k-inner: wh[p, s*KTD + k*D + c] = wt[e_s][k, p, c]
        whc = np.zeros((slots, KT, 128, D), np.float32)
        for s, e in enumerate(experts_on[c]):
            whc[s] = wt[e]
        whc = np.ascontiguousarray(whc.transpose(2, 1, 0, 3)).reshape(
            128, KT, slots * D).astype(_BF)
        slot_of = {e: s for s, e in enumerate(experts_on[c])}
        ohv = np.zeros(nblk, np.int32)
        for j, e in enumerate(core_blocks[c]):
            ohv[j] = slot_of[e] * D if e >= 0 else 0
        scc = ohv.reshape(1, nblk)
        hotc = np.concatenate(
            [whc[:, 0, 0:D], whc[:, 1, 0:D],
             xhc[:, 0:2, 0:256].reshape(128, 512)], axis=1)
        in_maps.append({"xh": xhc, "wh": whc, "sc": scc,
                        "hot": np.ascontiguousarray(hotc)})

    # combine positions for the host-side gather-add; dropped picks point
    # at a pad row (scale 0 -> exact zero)
    zeros_flat = np.flatnonzero(scales.reshape(-1) == 0.0)
    zeropos = int(zeros_flat[0]) if len(zeros_flat) else 0
    pos = np.empty((TOPK, B, L), np.int64)
    for bb in range(B):
        for i in range(TOPK):
            pp_ = pick_pos[(bb, i)]
            if pp_ is None:
                pos[i, bb] = zeropos
            else:
                e, rank = pp_
                pos[i, bb] = expert_rowpos[e][rank * L:(rank + 1) * L]
    return in_maps, pos, nblk, slots, nconst


LAST_RESULT = None


def kernel(cycle_curve_data, logits, moe_masks, W, b):
    global LAST_RESULT
    in_maps, pos, nblk, slots, nconst = make_in_maps(
        cycle_curve_data, logits, moe_masks, W, b)
    nc = get_nc(nblk=nblk, slots=slots, nconst=nconst)
    res = run_bass_kernel_spmd(nc, in_maps, core_ids=list(range(NCORES)))
    LAST_RESULT = res
    # out is [128m, nblk, D] per core; flat position space (c, j, m)
    flat = np.concatenate(
        [np.asarray(r["out"]).astype(np.float32).transpose(1, 0, 2)
         .reshape(nblk * 128, D) for r in res.results], axis=0)
    out = flat[pos[0].reshape(-1)] + flat[pos[1].reshape(-1)]
    return out.reshape(B, L, D).astype(_BF)


# revision 48
# speedup vs baseline: 1.0688x; 1.0650x over previous
"""Trainium2 Bass kernel for BatteryMoEFlattenIntraCycleMoELayer.

out[b] = sum_{e in top2(b)} gate[b,e] * (x[b] @ W_e.T + bias_e),  cast to bf16

Strategy: expert-packed dispatch with host-side routing.

The gate-weighted top-2 dispatch decomposes into ~251 (sample, expert) tasks
of shape [L=100, F=900] @ [900, D=512].  The host computes the gating, packs
the L-rows of all tasks routed to the same expert into dense 128-row blocks,
and balances the ~200 blocks across 8 cores (nblk slots each).  Each core
runs nblk x 8 dense bf16 matmuls (PSUM-accumulated over K=901 in 8 k-tiles),
scaling by the gate at PSUM->SBUF eviction; the host gathers each sample's
two partial blocks and adds them.

DMA cost model (validated in TimelineSim): every DMACopy serializes ~630ns
on a single HWDGE descriptor-generation slot regardless of size, transfers
serialize at ~360GB/s on the DMA-engine pool, and completion is signalled
~900ns after transfer end.  The baseline's 69 per-core DMAs made HWDGE a
co-bottleneck with the PE (43us each); this version merges to ~28 DMAs:

  - W in k-major layout [128, KT, slots*D]: slot-0 k-pairs paced with
    wave 0; the non-slot-0 chunks trail ALL x chunks (at real HW transfer
    rates they otherwise push the x13-17 chunk past the PE's block-13
    deadline, stalling the PE and resetting its p-state ramp).
  - x as one 3D tile [128, nblk, KT*128]: wave 0 (blocks 0-7) loaded as
    k-pair slices across all 8 blocks (k-outer matmul order consumes them
    in arrival order); remaining blocks in ~5-block chunks.
  - outputs staged in [128, 2, D] pair tiles; all pair stores issue
    after the last load, so store transfers never queue ahead of x
    arrivals in the DMA-engine FIFO (HW transfers run ~24% slower than
    the modeled 360GB/s, thinning the sim's slack).
  - W-slot offsets as a single [1, nblk] i32 row, loaded outside the
    early-wave transfer FIFO (gate scales are pre-folded into x on the
    host, so no per-block scale data ships at all).

DMAs are emitted just-in-time (right before their first consumer's pass) so
tile-framework hazard tracking never creates a dependency on a later-arriving
transfer.  A short junk-matmul warmup keeps the PE clock ramping while the
first DMAs land.
"""

import numpy as np
import ml_dtypes
from contextlib import ExitStack

import concourse.bass as bass
import concourse.bacc as bacc
import concourse.mybir as mybir
import concourse.tile as tile
from concourse.bass_utils import run_bass_kernel_spmd

# problem shape (hardcoded per contract)
B, L, C, CURVE = 128, 100, 3, 300
F = C * CURVE            # 900
E, D, TOPK = 8, 512, 2
EPS = 1e-9

NCORES = 8
KT = 8                   # contraction tiles of 128 (900+bias row padded to 1024)
FP = KT * 128            # 1024
KTD = KT * D             # per-slot W span in the s-major layout
NBLK_MAX = 26            # block-slot cap; sum_e ceil(100*n_e/128) <= 207
SLOTS_MAX = 4            # expert W-slot cap per core (packer asserts this)
WAVE = 8                 # blocks in flight (one PSUM bank each)
WARMUP_MMS = 26          # junk matmuls covering the first-DMA latency

BF16 = mybir.dt.bfloat16
F32 = mybir.dt.float32
I32 = mybir.dt.int32

_BF = ml_dtypes.bfloat16

_NC_CACHE = {}


def _chunks(start, end, step):
    return [(a, min(a + step, end)) for a in range(start, end, step)]


def _emit_body(nc, tc, ctx, xh, wh, sc, hot, out, nblk, slots, nconst,
               shared=None, R=""):
    PE = mybir.EngineType.PE

    gp = ctx.enter_context(tc.tile_pool(name=f"{R}gating", bufs=1))
    wp = ctx.enter_context(tc.tile_pool(name=f"{R}wpool", bufs=1))
    xp = ctx.enter_context(tc.tile_pool(name=f"{R}xpool", bufs=1))
    pp = ctx.enter_context(tc.tile_pool(name=f"{R}psum", bufs=WAVE, space="PSUM"))
    op = ctx.enter_context(
        tc.tile_pool(name=f"{R}outp", bufs=NBLK_MAX // 2 + 2))

    sc_t = gp.tile([1, nblk], I32, name=f"{R}sct")
    w_t = wp.tile([128, KT, slots * D], BF16, name=f"{R}wt")
    x_t = xp.tile([128, nblk, KT * 128], BF16, name=f"{R}xt")
    ht = wp.tile([128, 2 * D + 2 * 256], BF16, name=f"{R}ht")
    junk = wp.tile([128, 128], BF16, name=f"{R}junk")

    psum_t = {}
    for j in range(WAVE):
        psum_t[j] = pp.tile([128, D], F32, tag="ps", name=f"{R}ps{j}")

    # PE warmup: junk matmuls into psum bank 0 keep the PE clock ramping
    # while the first DMAs land; block 0's start=True k0 overwrites the
    # bank.  N=128 so each costs ~107ns mid-ramp.  Full memset: garbage
    # bf16 can be NaN/Inf on real hardware.
    nc.vector.memset(junk, 0)
    for _ in range(WARMUP_MMS):
        nc.tensor.matmul(psum_t[0][:, 0:128], junk, junk,
                         start=True, stop=True)

    sp = nc.sync

    # ---- wave-0 leading loads.  The hot buffer carries W-slot0-k01 and
    # x k01 of blocks 0-1 in ONE 3KB/partition DMA (one HWDGE slot + one
    # transfer), so the first matmul starts ~4.0us in.  k01 of blocks 0-1
    # (lhsT) and of all const-rhs blocks (rhs) are consumed from ht; the
    # w_t/x_t regions they would occupy are never loaded.
    first_rep = shared is None or "offs" not in shared
    sp.dma_start(ht, hot[:, :])
    sp.dma_start(x_t[:, 2:4, 0:256], xh[:, 2:4, 0:256])
    sp.dma_start(x_t[:, 4:WAVE, 0:256], xh[:, 4:WAVE, 0:256])
    if first_rep and nconst < WAVE:
        # dynamic rhs inside wave 0: offsets must land before k0
        sp.dma_start(sc_t, sc[:, :])

    # W-slot offsets are rep-invariant: load them into PE registers once
    # (rep 0) and reuse across repeats -- per-rep loads exhaust the 54-reg
    # PE file at high repeat counts.
    offs = [None] * nblk if first_rep else shared["offs"]

    def load_offs():
        if nconst >= nblk or not first_rep:
            return
        _, offs1 = nc.values_load_multi_w_load_instructions(
            sc_t[0:1, nconst:nblk], engines=(PE,),
            min_val=0, max_val=(slots - 1) * D,
            skip_runtime_bounds_check=True)
        offs[nconst:] = list(offs1)
        if shared is not None:
            shared["offs"] = offs

    if nconst < WAVE:
        load_offs()

    def emit_mm(j, k, ps=None, n0=0, n1=D):
        if j < nconst:
            rhs = (ht[:, k * D + n0:k * D + n1] if k < 2
                   else w_t[:, k, n0:n1])
        else:
            # ds on the last dim with element offsets (slot * D), the
            # register-liveness-friendly form
            off = offs[j] if n0 == 0 else offs[j] + n0
            rhs = w_t[:, k, bass.ds(off, n1 - n0)]
        if j < 2 and k < 2:
            lhs = ht[:, 2 * D + j * 256 + k * 128:2 * D + j * 256 + (k + 1) * 128]
        else:
            lhs = x_t[:, j, k * 128:(k + 1) * 128]
        nc.tensor.matmul(
            psum_t[j] if ps is None else ps, lhs, rhs,
            start=(k == 0), stop=(k == KT - 1))

    pair_t = {}
    deferred = []    # (p, j_hi): pair stores issued after ALL loads so
    #                  store transfers never delay x arrivals in the
    #                  DMA-engine FIFO (HW transfers run slower than the
    #                  sim's 360GB/s; the slack is thinner than modeled)

    def emit_evict(j):
        # gates are pre-folded into x on the host, so eviction is a pure
        # f32->bf16 convert (tensor_scalar_mul by 1.0, no sc dependency)
        p, h = divmod(j, 2)
        if h == 0:
            pair_t[p] = op.tile([128, 2, D], BF16, tag="ot", name=f"{R}ot{j}")
        nc.vector.tensor_scalar_mul(pair_t[p][:, h, :], psum_t[j], 1.0)
        if h == 1 or j == nblk - 2:
            deferred.append((p, j))

    # ---- wave 0: k-outer over blocks 0-7, with per-pass loads emitted
    # between passes (just-in-time emission keeps hazard tracking exact and
    # paces one ~630ns HWDGE slot per instruction).
    # Non-slot-0 W is needed from block `nconst` (~19 typical, >= 8 by
    # packer guarantee when possible); when nconst is small, load it before
    # the bulk x chunks instead of after.
    wrest_early = nconst < 14
    x_bulk = _chunks(WAVE, nblk, 5)
    wrest = []
    if slots > 1:
        wrest = [(0, KT // 2), (KT // 2, KT)]

    later_loads = []                     # emitted one per wave-0 pass
    for p in range(1, 4):                # k-pairs 23, 45, 67
        later_loads.append(("w0", (2 * p * D, (2 * p + 2) * D)))
        later_loads.append(("x0", (2 * p * 128, (2 * p + 2) * 128)))
    # w slot-0 k01 is consumed from ht by const-rhs blocks, but dynamic-rhs
    # blocks on cores whose slot-0 prefix exceeds the global nconst reach it
    # through w_t with offset 0 -- so load it (off the critical path).
    # At measured HW transfer rates the x13-17 chunk lands ~1.7us after
    # the PE's block-13 deadline when the W-rest chunks precede it in the
    # transfer FIFO.  W-rest isn't consumed until block nconst (>= 19
    # deadline ~37us), so in the normal case ALL x chunks go first and the
    # W-rest + dynamic-slot0 loads trail them.
    if wrest_early:
        bulk = ([("x", x_bulk[0])] +
                [("wr", rng) for rng in wrest] +
                [("w0", (0, 2 * D))] +
                [("x", rng) for rng in x_bulk[1:]])
    else:
        bulk = ([("x", rng) for rng in x_bulk] +
                [("wr", rng) for rng in wrest] +
                [("w0", (0, 2 * D))])
    later_loads += bulk

    def emit_load(item):
        eng = sp
        kind, (a, b) = item
        if kind == "w0":
            # a, b are column offsets within slot 0 across k-pairs: the
            # (p) pair loads k-tiles a//D..b//D of the slot-0 columns
            eng.dma_start(w_t[:, a // D:b // D, 0:D], wh[:, a // D:b // D, 0:D])
        elif kind == "wr":
            eng.dma_start(w_t[:, a:b, D:slots * D], wh[:, a:b, D:slots * D])
        elif kind == "x0":
            eng.dma_start(x_t[:, 0:WAVE, a:b], xh[:, 0:WAVE, a:b])
        else:
            eng.dma_start(x_t[:, a:b, :], xh[:, a:b, :])

    li = 0
    for k in range(KT):
        for j in range(WAVE):
            emit_mm(j, k)
        if li < len(later_loads):
            emit_load(later_loads[li])
            li += 1
        if k == 2 and first_rep and nconst >= WAVE:
            # tiny offsets row, kept out of the early-wave transfer FIFO
            sp.dma_start(sc_t, sc[:, :])
        if k == 3 and nconst >= WAVE:
            load_offs()
    # x chunks not yet emitted go out during the first steady blocks
    pending = later_loads[li:]

    # ---- steady state: evict the block whose PSUM bank is being recycled,
    # then run the next block k-inner.
    last = nblk - 1
    for j in range(WAVE, last):
        if pending:
            emit_load(pending.pop(0))
        emit_evict(j - WAVE)
        psum_t[j] = pp.tile([128, D], F32, tag="ps", name=f"{R}ps{j}")
        for k in range(KT):
            emit_mm(j, k)
    while pending:
        emit_load(pending.pop(0))

    # ---- final block: two half-width (N=256) matmul streams into TWO
    # psum banks, so half A's convert-evict + store overlap half B's
    # matmuls and the post-last-matmul chain is evict(392) -> issue ->
    # 182ns transfer -> sem.  The two recycled banks' drain evictions are
    # emitted first.
    H = D // 2
    emit_evict(last - WAVE)
    ps_a = pp.tile([128, H], F32, tag="ps", name=f"{R}psA")
    emit_evict(last - WAVE + 1)
    ps_b = pp.tile([128, H], F32, tag="ps", name=f"{R}psB")
    for k in range(KT):
        emit_mm(last, k, ps=ps_a, n0=0, n1=H)
    for j in range(last - WAVE + 2, last):
        emit_evict(j)
    for (p, j) in deferred:
        if j % 2 == 1:
            nc.scalar.dma_start(out[:, j - 1:j + 1, :], pair_t[p])
        else:
            # nblk even leaves block nblk-2 unpaired; store it solo
            nc.scalar.dma_start(out[:, j:j + 1, :], pair_t[p][:, 0:1, :])
    ot_l = op.tile([128, 1, D], BF16, tag="ot", name=f"{R}otl")
    nc.vector.tensor_scalar_mul(ot_l[:, 0, 0:H], ps_a, 1.0)
    nc.scalar.dma_start(out[:, last:last + 1, 0:H], ot_l[:, :, 0:H])
    for k in range(KT):
        emit_mm(last, k, ps=ps_b, n0=H, n1=D)
    nc.vector.tensor_scalar_mul(ot_l[:, 0, H:D], ps_b, 1.0)
    sp.dma_start(out[:, last:last + 1, H:D], ot_l[:, :, H:D])


def _build_nc(repeats=1, nblk=NBLK_MAX, slots=SLOTS_MAX, nconst=0):
    nc = bacc.Bacc("TRN2", target_bir_lowering=False)

    xh = nc.declare_dram_parameter("xh", [128, nblk, KT * 128], BF16,
                                   isOutput=False)
    wh = nc.declare_dram_parameter("wh", [128, KT, slots * D], BF16,
                                   isOutput=False)
    sc = nc.declare_dram_parameter("sc", [1, nblk], I32, isOutput=False)
    hot = nc.declare_dram_parameter("hot", [128, 2 * D + 2 * 256], BF16,
                                    isOutput=False)
    out = nc.declare_dram_parameter("out", [128, nblk, D], BF16, isOutput=True)

    with tile.TileContext(nc) as tc, ExitStack() as ctx:
        shared = {}
        for rep in range(repeats):
            R = f"r{rep}_" if repeats > 1 else ""
            with ExitStack() as rctx:
                _emit_body(nc, tc, rctx, xh, wh, sc, hot, out,
                           nblk, slots, nconst, shared=shared, R=R)

    nc.compile()
    return nc


def get_nc(repeats=1, nblk=NBLK_MAX, slots=SLOTS_MAX, nconst=0):
    key = ("nc", repeats, nblk, slots, nconst)
    if key not in _NC_CACHE:
        _NC_CACHE[key] = _build_nc(repeats, nblk, slots, nconst)
    return _NC_CACHE[key]


def _host_gates(logits, moe_masks):
    """Reference gating on host -> per-sample (g0, g1), (e0, e1)."""
    lg = np.asarray(logits, np.float64)
    mk = (np.asarray(moe_masks, np.int64) == 1).astype(np.float64)
    p = np.exp(lg - lg.max(axis=1, keepdims=True))
    p /= p.sum(axis=1, keepdims=True)
    g = p * mk                                              # [B, E]
    idx = np.argsort(-g, axis=1, kind="stable")[:, :TOPK]   # top-2 indices
    gv = np.take_along_axis(g, idx, axis=1)                 # [B, 2]
    gv = gv / (gv.sum(axis=1, keepdims=True) + EPS)         # renormalize
    return gv.astype(np.float32), idx.astype(np.int64)


def _assign_blocks(nblocks_per_expert, nblk):
    """Distribute each expert's blocks over 8 cores of nblk slots,
    minimizing distinct experts per core.  Phase 1: every expert gets its
    own (empty) core, largest first, filled up to nblk.  Phase 2: leftover
    pieces go to the cores with the fewest distinct experts / most room."""
    cap = [nblk] * NCORES
    experts_on = [[] for _ in range(NCORES)]   # ordered distinct experts
    placed = [[] for _ in range(NCORES)]       # (expert, nblocks)

    def put(c, e, take):
        cap[c] -= take
        if e not in experts_on[c]:
            experts_on[c].append(e)
        placed[c].append((e, take))

    order = [e for e in sorted(range(E), key=lambda e: -nblocks_per_expert[e])
             if nblocks_per_expert[e] > 0]
    leftovers = []
    nxt = 0
    for e in order:
        rem = nblocks_per_expert[e]
        if nxt < NCORES:
            take = min(rem, nblk)
            put(nxt, e, take)
            nxt += 1
            rem -= take
        if rem:
            leftovers.append((e, rem))
    leftovers.sort(key=lambda x: -x[1])
    for e, rem in leftovers:
        while rem > 0:
            cands = [c for c in range(NCORES) if cap[c] > 0]
            cands.sort(key=lambda c: (e not in experts_on[c],
                                      len(experts_on[c]), -cap[c]))
            c = cands[0]
            take = min(rem, cap[c])
            put(c, e, take)
            rem -= take
    nslots = max(len(x) for x in experts_on)
    assert nslots <= SLOTS_MAX, (
        f"packing needs {nslots} experts on one core > {SLOTS_MAX}")
    return placed, experts_on, max(2, nslots)


def _prep_w_full(W, b):
    """-> [E, KT, 128, D] f32 k-tiled transposed-padded weights."""
    wt = np.zeros((E, FP, D), np.float32)
    wt[:, :F, :] = np.asarray(W, np.float32).transpose(0, 2, 1)
    wt[:, F, :] = np.asarray(b, np.float32)
    return wt.reshape(E, KT, 128, D)


def make_in_maps(cycle_curve_data, logits, moe_masks, W, b):
    gv, idx = _host_gates(logits, moe_masks)

    # per-expert routed sample lists (zero-gate picks contribute exactly 0
    # and are dropped from dispatch; their combine position points at a
    # guaranteed-zero pad row)
    samples_e = [[] for _ in range(E)]     # (sample, gate)
    pick_pos = {}                          # (b, i) -> (expert, rank) | None
    for bb in range(B):
        for i in range(TOPK):
            e = int(idx[bb, i])
            g = float(gv[bb, i])
            if g == 0.0:
                pick_pos[(bb, i)] = None
                continue
            pick_pos[(bb, i)] = (e, len(samples_e[e]))
            samples_e[e].append((bb, g))
    n_e = [len(s) for s in samples_e]
    B_e = [int(np.ceil(L * n / 128)) if n else 0 for n in n_e]
    nblk = max(WAVE, int(np.ceil(sum(B_e) / NCORES)))
    assert nblk <= NBLK_MAX

    placed, _, slots = _assign_blocks(B_e, nblk)

    # Per-core block order: the core's largest expert becomes W-slot 0 and
    # its blocks (plus any pad blocks, which are also slot-0/offset-0) come
    # first, so a compile-time-constant rhs covers the first nconst blocks.
    experts_on = [[] for _ in range(NCORES)]
    core_blocks = [[] for _ in range(NCORES)]  # expert id per slot, -1 pad
    nconst = nblk
    for c in range(NCORES):
        cnt = {}
        for (e, take) in placed[c]:
            cnt[e] = cnt.get(e, 0) + take
        exps = sorted(cnt, key=lambda e: -cnt[e])
        experts_on[c] = exps
        npads = nblk - sum(cnt.values())
        if exps:
            seq = [exps[0]] * cnt[exps[0]] + [-1] * npads
            for e in exps[1:]:
                seq += [e] * cnt[e]
            nconst = min(nconst, cnt[exps[0]] + npads)
        else:
            seq = [-1] * nblk
        core_blocks[c] = seq

    # global row stream per expert -> (core, slot j, partition m) positions
    # flat position space: core*nblk*128 + j*128 + m
    expert_rowpos = {}                     # e -> int64 [100*n_e]
    next_blk_of = [0] * E
    expert_block_flat = [np.empty(B_e[e], np.int64) for e in range(E)]
    for c in range(NCORES):
        for j, e in enumerate(core_blocks[c]):
            if e >= 0:
                expert_block_flat[e][next_blk_of[e]] = c * nblk + j
                next_blk_of[e] += 1
    for e in range(E):
        if n_e[e] == 0:
            continue
        r = np.arange(L * n_e[e], dtype=np.int64)
        expert_rowpos[e] = expert_block_flat[e][r // 128] * 128 + r % 128

    # ---- pack x: xr[(b,l), f] = x row-major, padded to 1024 with ones@900
    # (kept f32; the per-row gate is folded in at the per-core gather)
    x = np.asarray(cycle_curve_data, np.float32).reshape(B, L, F)
    xr = np.zeros((B * L, FP), np.float32)
    xr[:, :F] = x.reshape(B * L, F)
    xr[:, F] = 1.0

    # per-core row index [nblk*128] into xr (pad rows -> 0 with scale 0)
    rowidx = np.zeros((NCORES, nblk * 128), np.int64)
    scales = np.zeros((NCORES, nblk * 128), np.float32)
    for e in range(E):
        if n_e[e] == 0:
            continue
        src = np.empty(L * n_e[e], np.int64)    # xr row ids of this stream
        gts = np.empty(L * n_e[e], np.float32)
        for r, (bb, g) in enumerate(samples_e[e]):
            src[r * L:(r + 1) * L] = np.arange(bb * L, (bb + 1) * L)
            gts[r * L:(r + 1) * L] = g
        pos = expert_rowpos[e]
        c = pos // (nblk * 128)
        m = pos % (nblk * 128)
        rowidx[c, m] = src
        scales[c, m] = gts

    # gather + transpose to device layout
    wt = _prep_w_full(W, b)
    in_maps = []
    for c in range(NCORES):
        xb = (xr[rowidx[c]] * scales[c][:, None]).astype(_BF)
        xb = xb.reshape(nblk, 128, KT, 128)         # [j, m, k, p]
        xhc = np.ascontiguousarray(xb.transpose(3, 0, 2, 1)).reshape(
            128, nblk, KT * 128)
        # W s-major k-inner: wh[p, s*KTD + k*D + c] = wt[e_s][k, p, c]
        whc = np.zeros((slots, KT, 128, D), np.float32)
        for s, e in enumerate(experts_on[c]):
            whc[s] = wt[e]
        whc = np.ascontiguousarray(whc.transpose(2, 1, 0, 3)).reshape(
            128, KT, slots * D).astype(_BF)
        slot_of = {e: s for s, e in enumerate(experts_on[c])}
        ohv = np.zeros(nblk, np.int32)
        for j, e in enumerate(core_blocks[c]):
            ohv[j] = slot_of[e] * D if e >= 0 else 0
        scc = ohv.reshape(1, nblk)
        hotc = np.concatenate(
            [whc[:, 0, 0:D], whc[:, 1, 0:D],
             xhc[:, 0:2, 0:256].reshape(128, 512)], axis=1)
        in_maps.append({"xh": xhc, "wh": whc, "sc": scc,
                        "hot": np.ascontiguousarray(hotc)})

    # combine positions for the host-side gather-add; dropped picks point
    # at a pad row (scale 0 -> exact zero)
    zeros_flat = np.flatnonzero(scales.reshape(-1) == 0.0)
    zeropos = int(zeros_flat[0]) if len(zeros_flat) else 0
    pos = np.empty((TOPK, B, L), np.int64)
    for bb in range(B):
        for i in range(TOPK):
            pp_ = pick_pos[(bb, i)]
            if pp_ is None:
                pos[i, bb] = zeropos
            else:
                e, rank = pp_
                pos[i, bb] = expert_rowpos[e][rank * L:(rank + 1) * L]
    return in_maps, pos, nblk, slots, nconst


LAST_RESULT = None


def kernel(cycle_curve_data, logits, moe_masks, W, b):
    global LAST_RESULT
    in_maps, pos, nblk, slots, nconst = make_in_maps(
        cycle_curve_data, logits, moe_masks, W, b)
    nc = get_nc(nblk=nblk, slots=slots, nconst=nconst)
    res = run_bass_kernel_spmd(nc, in_maps, core_ids=list(range(NCORES)))
    LAST_RESULT = res
    # out is [128m, nblk, D] per core; flat position space (c, j, m)
    flat = np.concatenate(
        [np.asarray(r["out"]).astype(np.float32).transpose(1, 0, 2)
         .reshape(nblk * 128, D) for r in res.results], axis=0)
    out = flat[pos[0].reshape(-1)] + flat[pos[1].reshape(-1)]
    return out.reshape(B, L, D).astype(_BF)
